# revision 12
# baseline (speedup 1.0000x reference)
"""Trainium2 Bass kernel for nn_BilinearPairedLayer (v2).

out[b,i,j,o] = hl[b,i] @ W[o] @ hr[b,j] + bb[o]
  hl = celu(zl @ fc_l_W^T + fc_l_b), hr = celu(zr @ fc_r_W^T + fc_r_b)
  zl = [x_l, x_l[i-1], x_l[i+1]], zr = [x_l, x_r[j+1], x_r[j-1]]  (192 feats;
  zr[0]=x_l reproduces the torch-source init bug).

Shapes: B=2, N=512, n_in=64, H=128, n_out=8 -> out [2,512,512,8] f32.
Sharding: 8 cores = (b in {0,1}) x (j-chunk in {0..3} of 128 columns).

v2 design notes (vs the 28.9us v1):
- 64-deep contraction packing: each layer-1 shift contracts only 64
  features, so two shifts share one 128-partition matmul (stationary
  [w_c1; w_c0], moving [x_c1; x_c0]); the third shift rides a stationary
  with a zeroed lower half that kills the garbage moving rows.  6 matmuls
  -> 4, and 1920 -> 1280 PE columns.
- celu(z) = min(exp(z)-1, relu(z)) == max(z,0)+min(exp(z)-1,0): one DVE
  tensor_scalar (relu+bias), one ACT exp (bias fused), one DVE
  scalar_tensor_tensor; ~3 ops instead of 4 with a shorter chain.
- PE p-state ramp: the tensor engine runs 0.65/1.2/2.4 GHz (cold/warm/
  >=3us continuously busy).  Dummy matmuls on zeroed scratch fill the
  input-DMA wait so real matmuls inherit a hot clock.
- bf16 output + host-side bias add: halves the output DMA bytes and turns
  evictions into pure psum->bf16 casts that spread over DVE/ACT/GpSimd.
- og-major output layout (col = og*512 + j*4 + ol): a main matmul only
  waits on its own o-group's v2 cast; host transposes back.
- input split across both HWDGE rings: sync ring Da (weights+hr moving)
  then Wt; ACT ring Db (hl moving); output per-ic [128,1024]bf16 tiles
  alternate rings.

walrus's per-instruction HW structs carry at most ONE sync wait; a
post-pass splits multi-wait instructions into single-wait EventSemaphore
predecessors.
"""

import os

import numpy as np

import concourse.bass as bass
import concourse.mybir as mybir
import concourse.tile as tile
from concourse.bass_utils import run_bass_kernel_spmd

F32 = mybir.dt.float32
BF16 = mybir.dt.bfloat16
ADD = mybir.AluOpType.add
MAX = mybir.AluOpType.max
MIN = mybir.AluOpType.min

B = 2
N = 512
NIN = 64
H = 128
O = 8
JC = 128  # j-chunk per core
N_CORES = 8

N_WARM = int(os.environ.get("BK_WARM", "9"))

# Da column offsets (bf16).  Bias cols hold raw f32 bytes (bitcast back).
_S1_HR = 0      # [frT_1 ; frT_0]
_S2_HR = 128    # [frT_2 ; 0]
_S1_HL = 256    # [flT_1 ; flT_0]
_S2_HL = 384    # [flT_2 ; 0]
_RJ = 512       # rows 0:64 xr window, rows 64:128 xlj (132 cols)
_BR = 644       # fc_r_b (2 cols raw f32)
_BL = 646       # fc_l_b
_DA_W = 648
_DB_W = 516     # rows 0:64 x_l at cols 2:514, rows 64:128 x_l at cols 1:513


def build_nc():
    nc = bass.Bass("TRN2")

    Da_d = nc.dram_tensor("Da", [128, _DA_W], BF16, kind="ExternalInput")
    Db_d = nc.dram_tensor("Db", [128, _DB_W], BF16, kind="ExternalInput")
    Wt_d = nc.dram_tensor("Wt", [128, O * H], BF16, kind="ExternalInput")
    out_d = nc.dram_tensor("out", [N, JC * O], BF16, kind="ExternalOutput")

    with tile.TileContext(nc) as tc:
        with (
            tc.tile_pool(name="persist", bufs=1) as pp,
            tc.tile_pool(name="scratch", bufs=2) as sp,
            tc.tile_pool(name="outbuf", bufs=4) as op,
            tc.tile_pool(name="ps_l1", bufs=1, space="PSUM") as ps_l1,
            tc.tile_pool(name="ps_v", bufs=2, space="PSUM") as ps_v,
            tc.tile_pool(name="ps_main", bufs=4, space="PSUM") as ps_main,
        ):
            # zeroed scratch: PE pre-ramp matmul fodder + ACT warm src
            sc = pp.tile([128, 256], BF16, name="sc")
            nc.vector.memset(sc[:], 0.0)
            td = pp.tile([1, 2], F32, name="td")
            nc.vector.memset(td[0:1, 0:1], 0.0)

            Da_sb = pp.tile([128, _DA_W], BF16, name="Da_sb")
            Db_sb = pp.tile([128, _DB_W], BF16, name="Db_sb")
            W_sb = pp.tile([128, O * H], BF16, name="W_sb")
            # Da rows split across both rings so its receipt lands ~600ns
            # earlier (halves the per-engine descriptor count + tail spread).
            nc.sync.dma_start(Da_sb[0:64, :], Da_d[0:64, :])
            nc.scalar.dma_start(Da_sb[64:128, :], Da_d[64:128, :])
            nc.sync.dma_start(W_sb[:], Wt_d[:])
            nc.scalar.dma_start(Db_sb[:], Db_d[:])
            # lazy PWP activation-table load (~1.3us) after the ACT-ring
            # DMA issue, during the input-DMA wait.
            nc.scalar.activation(td[0:1, 1:2], td[0:1, 0:1],
                                 mybir.ActivationFunctionType.Exp)

            br_ap = Da_sb[:, _BR:_BR + 2].bitcast(F32)
            bl_ap = Da_sb[:, _BL:_BL + 2].bitcast(F32)

            # ---- PE pre-ramp: dummy matmuls on zeros ----
            ps_w = ps_main.tile([128, 512], F32, name="ps_m")
            for _ in range(N_WARM):
                nc.tensor.matmul(ps_w[:, 0:256], sc[:, 0:128], sc[:],
                                 start=True, stop=True)

            # ---- layer 1 matmuls (64-deep packed) ----
            # hr[g,j]: mm1 = frT_1.T @ xr[j+1] (+) frT_0.T @ xlj;
            #          mm2 = frT_2.T @ xr[j-1] (lower stationary half = 0)
            ps_hr = ps_l1.tile([128, JC], F32, name="ps_hr")
            nc.tensor.matmul(ps_hr[:], Da_sb[:, _S1_HR:_S1_HR + 128],
                             Da_sb[:, _RJ + 2:_RJ + 2 + JC],
                             start=True, stop=False)
            nc.tensor.matmul(ps_hr[:], Da_sb[:, _S2_HR:_S2_HR + 128],
                             Da_sb[:, _RJ:_RJ + JC],
                             start=False, stop=True)
            # hl[h,i]: mm1 = flT_1.T @ x[i-1] (+) flT_0.T @ x[i];
            #          mm2 = flT_2.T @ x[i+1]
            ps_hl = ps_l1.tile([128, N], F32, name="ps_hl")
            nc.tensor.matmul(ps_hl[:], Da_sb[:, _S1_HL:_S1_HL + 128],
                             Db_sb[:, 1:1 + N], start=True, stop=False)
            nc.tensor.matmul(ps_hl[:], Da_sb[:, _S2_HL:_S2_HL + 128],
                             Db_sb[:, 3:3 + N], start=False, stop=True)

            # ---- celu: h = min(exp(z)-1, relu(z)), z = psum + bias ----
            # (GpSimd cannot touch PSUM, so relu goes on DVE or ACT.)
            def celu(ps_ap, bias_ap, h_ap, w, tag, r_on_act=False):
                # persistent tiles: sp-pool buffer reuse creates false WAR
                # serialization between the r/e streams of adjacent celus
                r = pp.tile([128, w], F32, name=f"r_{tag}")
                e = pp.tile([128, w], F32, name=f"e_{tag}")
                if r_on_act:
                    nc.scalar.activation(r[:], ps_ap,
                                         mybir.ActivationFunctionType.Relu,
                                         bias=bias_ap, scale=1.0)
                else:
                    nc.vector.tensor_scalar(r[:], ps_ap, bias_ap, 0.0,
                                            ADD, MAX)
                nc.scalar.activation(e[:], ps_ap,
                                     mybir.ActivationFunctionType.Exp,
                                     bias=bias_ap, scale=1.0)
                nc.vector.scalar_tensor_tensor(h_ap, e[:], -1.0, r[:],
                                               ADD, MIN)

            hrT = pp.tile([128, JC], BF16, name="hrT")
            celu(ps_hr[:], br_ap, hrT[:], JC, "hr")
            hlT = pp.tile([128, N], BF16, name="hlT")
            celu(ps_hl[:, 0:256], bl_ap, hlT[:, 0:256], 256, "hl0")

            # ---- v2[h, (j,o)] og-major ----
            v2f = [pp.tile([128, 512], BF16, name=f"v2f{og}")
                   for og in range(2)]
            ps_vs = []
            for og in range(2):
                ps_vo = ps_v.tile([128, 512], F32, name="ps_vo")
                ps_vs.append(ps_vo)
                for ol in range(4):
                    o = og * 4 + ol
                    nc.tensor.matmul(
                        ps_vo[:, ol * JC:(ol + 1) * JC],
                        W_sb[:, o * H:(o + 1) * H], hrT[:],
                        start=True, stop=True,
                    )

            def cast_og(og):
                src = ps_vs[og][:].rearrange("p (o j) -> p j o", o=4)
                dst = v2f[og][:].rearrange("p (j o) -> p j o", o=4)
                nc.vector.tensor_copy(dst[:, 0:64, :], src[:, 0:64, :])
                nc.scalar.copy(dst[:, 64:128, :], src[:, 64:128, :])

            cast_og(0)
            celu(ps_hl[:, 256:512], bl_ap, hlT[:, 256:512], 256, "hl1")
            cast_og(1)

            # ---- main: out[i, og*512 + j*4 + ol] ----
            evict = [nc.vector.tensor_copy, nc.scalar.copy]
            out_sbs = {}
            for k in range(8):
                ic, og = k // 2, k % 2
                if og == 0:
                    out_sbs[ic] = op.tile([128, 1024], BF16, name="out_sb")
                out_sb = out_sbs[ic]
                ps_m = ps_main.tile([128, 512], F32, name="ps_m")
                nc.tensor.matmul(
                    ps_m[:], hlT[:, ic * 128:(ic + 1) * 128], v2f[og][:],
                    start=True, stop=True,
                )
                evict[k % 2](out_sb[:, og * 512:(og + 1) * 512], ps_m[:])
                if og == 1:
                    # sync ring: ACT is elementwise-loaded, sync is idle
                    nc.sync.dma_start(out_d[ic * 128:(ic + 1) * 128, :],
                                      out_sb[:])

    _legalize_waits(nc)
    return nc


def _legalize_waits(nc):
    """walrus's per-instruction HW structs carry at most ONE sync wait.
    Split any instruction with >1 on_wait into same-engine single-wait
    EventSemaphore predecessors (engine executes them in program order)."""
    n = 0
    for bb in nc.main_func.blocks:
        insts = list(bb.instructions)
        out = []
        for ins in insts:
            si = ins.sync_info
            waits = list(si.on_wait) if si and si.on_wait else []
            if len(waits) > 1:
                for w in waits[:-1]:
                    n += 1
                    out.append(mybir.InstEventSemaphore(
                        name=f"wait-split-{n}",
                        opcode="EventSemaphore",
                        engine=ins.engine,
                        ins=[], outs=[],
                        sync_info=mybir.SyncInfo(on_wait=[w], on_update=[]),
                    ))
                si.on_wait = [waits[-1]]
            out.append(ins)
        if n:
            bb.instructions = out
    return nc


_NC_CACHE = None


def _get_nc():
    global _NC_CACHE
    if _NC_CACHE is None:
        _NC_CACHE = build_nc()
    return _NC_CACHE


def _prep_core_inputs(x_l, x_r, fc_l_W, fc_l_b, fc_r_W, fc_r_b,
                      bilinear_W, bilinear_b):
    """Host-side sharding: build the 8 per-core input dicts."""
    import ml_dtypes

    f32 = np.float32
    bf16 = ml_dtypes.bfloat16
    x_l = np.ascontiguousarray(x_l, f32)
    x_r = np.ascontiguousarray(x_r, f32)
    flW = np.asarray(fc_l_W, f32)
    frW = np.asarray(fc_r_W, f32)

    # WT[g, o*H + h] = W[o, h, g]
    WT = np.ascontiguousarray(
        np.asarray(bilinear_W, f32).transpose(2, 0, 1).reshape(128, O * H)
    ).astype(bf16)

    # Da: stationaries + hr moving + biases (identical weights per core)
    Da0 = np.zeros((128, _DA_W), bf16)

    def stat(w, c_hi, c_lo):
        """[w_{c_hi}.T ; w_{c_lo}.T] as [128,128] bf16 (None -> zeros)."""
        s = np.zeros((128, 128), f32)
        if c_hi is not None:
            s[:NIN] = w[:, c_hi * NIN:(c_hi + 1) * NIN].T
        if c_lo is not None:
            s[NIN:] = w[:, c_lo * NIN:(c_lo + 1) * NIN].T
        return s.astype(bf16)

    Da0[:, _S1_HR:_S1_HR + 128] = stat(frW, 1, 0)
    Da0[:, _S2_HR:_S2_HR + 128] = stat(frW, 2, None)
    Da0[:, _S1_HL:_S1_HL + 128] = stat(flW, 1, 0)
    Da0[:, _S2_HL:_S2_HL + 128] = stat(flW, 2, None)
    u2 = Da0.view(np.uint16)
    u2[:, _BR:_BR + 2] = np.asarray(fc_r_b, f32).reshape(-1, 1).view('<u2')
    u2[:, _BL:_BL + 2] = np.asarray(fc_l_b, f32).reshape(-1, 1).view('<u2')

    # Db per batch: rows 0:64 x_l at col offset 2, rows 64:128 at offset 1
    Dbs = []
    for b in range(B):
        Db = np.zeros((128, _DB_W), bf16)
        Db[:NIN, 2:2 + N] = x_l[b].T.astype(bf16)
        Db[NIN:, 1:1 + N] = x_l[b].T.astype(bf16)
        Dbs.append(Db)

    in_maps = []
    for core in range(N_CORES):
        b, jg = core // 4, core % 4
        j0 = jg * JC
        Da = Da0.copy()
        # rows 0:64: col _RJ+k = x_r[b, j0-1+k], k=0..129, zero-clipped
        lo = max(j0 - 1, 0)
        hi = min(j0 + JC + 1, N)
        Da[:NIN, _RJ + lo - (j0 - 1):_RJ + hi - (j0 - 1)] = \
            x_r[b, lo:hi].T.astype(bf16)
        # rows 64:128: col _RJ+2+j = x_l[b, j0+j]
        Da[NIN:, _RJ + 2:_RJ + 2 + JC] = \
            x_l[b, j0:j0 + JC].T.astype(bf16)
        in_maps.append({"Da": Da, "Db": Dbs[b], "Wt": WT})
    return in_maps


def _run(inputs, trace=False, **kw):
    nc = _get_nc()
    in_maps = _prep_core_inputs(**inputs)
    res = run_bass_kernel_spmd(
        nc, in_maps, core_ids=list(range(N_CORES)), trace=trace, **kw)
    bb = np.asarray(inputs["bilinear_b"], np.float32)
    out = np.empty((B, N, N, O), np.float32)
    for core in range(N_CORES):
        b, jg = core // 4, core % 4
        j0 = jg * JC
        # device col = og*512 + j*4 + ol  ->  [i, j, og*4+ol]
        arr = np.asarray(res.results[core]["out"]).astype(np.float32)
        arr = arr.reshape(N, 2, JC, 4).transpose(0, 2, 1, 3).reshape(N, JC, O)
        out[b, :, j0:j0 + JC, :] = arr + bb
    return out, res


def kernel(**inputs):
    out, _ = _run(inputs, trace=False)
    return out


# revision 14
# speedup vs baseline: 1.0356x; 1.0356x over previous
"""Trainium2 Bass kernel for nn_BilinearPairedLayer (v2).

out[b,i,j,o] = hl[b,i] @ W[o] @ hr[b,j] + bb[o]
  hl = celu(zl @ fc_l_W^T + fc_l_b), hr = celu(zr @ fc_r_W^T + fc_r_b)
  zl = [x_l, x_l[i-1], x_l[i+1]], zr = [x_l, x_r[j+1], x_r[j-1]]  (192 feats;
  zr[0]=x_l reproduces the torch-source init bug).

Shapes: B=2, N=512, n_in=64, H=128, n_out=8 -> out [2,512,512,8] f32.
Sharding: 8 cores = (b in {0,1}) x (j-chunk in {0..3} of 128 columns).

v2 design notes (vs the 28.9us v1):
- 64-deep contraction packing: each layer-1 shift contracts only 64
  features, so two shifts share one 128-partition matmul (stationary
  [w_c1; w_c0], moving [x_c1; x_c0]); the third shift rides a stationary
  with a zeroed lower half that kills the garbage moving rows.  6 matmuls
  -> 4, and 1920 -> 1280 PE columns.
- celu(z) = min(exp(z)-1, relu(z)) == max(z,0)+min(exp(z)-1,0): one DVE
  tensor_scalar (relu+bias), one ACT exp (bias fused), one DVE
  scalar_tensor_tensor; ~3 ops instead of 4 with a shorter chain.
- PE p-state ramp: the tensor engine runs 0.65/1.2/2.4 GHz (cold/warm/
  >=3us continuously busy).  Dummy matmuls on zeroed scratch fill the
  input-DMA wait so real matmuls inherit a hot clock.
- bf16 output + host-side bias add: halves the output DMA bytes and turns
  evictions into pure psum->bf16 casts that spread over DVE/ACT/GpSimd.
- og-major output layout (col = og*512 + j*4 + ol): a main matmul only
  waits on its own o-group's v2 cast; host transposes back.
- input split across both HWDGE rings: sync ring Da (weights+hr moving)
  then Wt; ACT ring Db (hl moving); output per-ic [128,1024]bf16 tiles
  alternate rings.

walrus's per-instruction HW structs carry at most ONE sync wait; a
post-pass splits multi-wait instructions into single-wait EventSemaphore
predecessors.
"""

import os

import numpy as np

import concourse.bass as bass
import concourse.mybir as mybir
import concourse.tile as tile
from concourse.bass_utils import run_bass_kernel_spmd

F32 = mybir.dt.float32
BF16 = mybir.dt.bfloat16
ADD = mybir.AluOpType.add
MAX = mybir.AluOpType.max
MIN = mybir.AluOpType.min

B = 2
N = 512
NIN = 64
H = 128
O = 8
JC = 128  # j-chunk per core
N_CORES = 8

N_WARM = int(os.environ.get("BK_WARM", "9"))

# Da column offsets (bf16).  Bias cols hold raw f32 bytes (bitcast back).
_S1_HR = 0      # [frT_1 ; frT_0]
_S2_HR = 128    # [frT_2 ; 0]
_S1_HL = 256    # [flT_1 ; flT_0]
_S2_HL = 384    # [flT_2 ; 0]
_RJ = 512       # rows 0:64 xr window, rows 64:128 xlj (132 cols)
_BR = 644       # fc_r_b (2 cols raw f32)
_BL = 646       # fc_l_b
_DA_W = 648
_DB_W = 516     # rows 0:64 x_l at cols 2:514, rows 64:128 x_l at cols 1:513


def build_nc():
    nc = bass.Bass("TRN2")

    Da_d = nc.dram_tensor("Da", [128, _DA_W], BF16, kind="ExternalInput")
    Db_d = nc.dram_tensor("Db", [128, _DB_W], BF16, kind="ExternalInput")
    Wt_d = nc.dram_tensor("Wt", [128, O * H], BF16, kind="ExternalInput")
    out_d = nc.dram_tensor("out", [N, JC * O], BF16, kind="ExternalOutput")

    with tile.TileContext(nc) as tc:
        with (
            tc.tile_pool(name="persist", bufs=1) as pp,
            tc.tile_pool(name="scratch", bufs=2) as sp,
            tc.tile_pool(name="outbuf", bufs=4) as op,
            tc.tile_pool(name="ps_l1", bufs=1, space="PSUM") as ps_l1,
            tc.tile_pool(name="ps_v", bufs=2, space="PSUM") as ps_v,
            tc.tile_pool(name="ps_main", bufs=4, space="PSUM") as ps_main,
        ):
            # zeroed scratch: PE pre-ramp matmul fodder + ACT warm src
            sc = pp.tile([128, 256], BF16, name="sc")
            nc.vector.memset(sc[:], 0.0)
            td = pp.tile([1, 2], F32, name="td")
            nc.vector.memset(td[0:1, 0:1], 0.0)

            Da_sb = pp.tile([128, _DA_W], BF16, name="Da_sb")
            Db_sb = pp.tile([128, _DB_W], BF16, name="Db_sb")
            W_sb = pp.tile([128, O * H], BF16, name="W_sb")
            # one DMA per ring first (the 2nd DMA on a ring pays ~1us of
            # serialized descriptor-generation latency): Da=sync, Db=ACT
            nc.sync.dma_start(Da_sb[:], Da_d[:])
            nc.scalar.dma_start(Db_sb[:], Db_d[:])
            nc.sync.dma_start(W_sb[:], Wt_d[:])
            # lazy PWP activation-table load (~1.3us) after the ACT-ring
            # DMA issue, during the input-DMA wait.
            nc.scalar.activation(td[0:1, 1:2], td[0:1, 0:1],
                                 mybir.ActivationFunctionType.Exp)

            br_ap = Da_sb[:, _BR:_BR + 2].bitcast(F32)
            bl_ap = Da_sb[:, _BL:_BL + 2].bitcast(F32)

            # ---- PE pre-ramp: dummy matmuls on zeros ----
            ps_w = ps_main.tile([128, 512], F32, name="ps_m")
            for _ in range(N_WARM):
                nc.tensor.matmul(ps_w[:, 0:256], sc[:, 0:128], sc[:],
                                 start=True, stop=True)

            # ---- layer 1 matmuls (64-deep packed) ----
            # hr[g,j]: mm1 = frT_1.T @ xr[j+1] (+) frT_0.T @ xlj;
            #          mm2 = frT_2.T @ xr[j-1] (lower stationary half = 0)
            ps_hr = ps_l1.tile([128, JC], F32, name="ps_hr")
            nc.tensor.matmul(ps_hr[:], Da_sb[:, _S1_HR:_S1_HR + 128],
                             Da_sb[:, _RJ + 2:_RJ + 2 + JC],
                             start=True, stop=False)
            nc.tensor.matmul(ps_hr[:], Da_sb[:, _S2_HR:_S2_HR + 128],
                             Da_sb[:, _RJ:_RJ + JC],
                             start=False, stop=True)
            # hl[h,i]: mm1 = flT_1.T @ x[i-1] (+) flT_0.T @ x[i];
            #          mm2 = flT_2.T @ x[i+1]
            ps_hl = ps_l1.tile([128, N], F32, name="ps_hl")
            nc.tensor.matmul(ps_hl[:], Da_sb[:, _S1_HL:_S1_HL + 128],
                             Db_sb[:, 1:1 + N], start=True, stop=False)
            nc.tensor.matmul(ps_hl[:], Da_sb[:, _S2_HL:_S2_HL + 128],
                             Db_sb[:, 3:3 + N], start=False, stop=True)

            # ---- celu: h = min(exp(z)-1, relu(z)), z = psum + bias ----
            # (GpSimd cannot touch PSUM, so relu goes on DVE or ACT.)
            def celu(ps_ap, bias_ap, h_ap, w, tag, r_on_act=False):
                # persistent tiles: sp-pool buffer reuse creates false WAR
                # serialization between the r/e streams of adjacent celus
                r = pp.tile([128, w], F32, name=f"r_{tag}")
                e = pp.tile([128, w], F32, name=f"e_{tag}")
                if r_on_act:
                    nc.scalar.activation(r[:], ps_ap,
                                         mybir.ActivationFunctionType.Relu,
                                         bias=bias_ap, scale=1.0)
                else:
                    nc.vector.tensor_scalar(r[:], ps_ap, bias_ap, 0.0,
                                            ADD, MAX)
                nc.scalar.activation(e[:], ps_ap,
                                     mybir.ActivationFunctionType.Exp,
                                     bias=bias_ap, scale=1.0)
                nc.vector.scalar_tensor_tensor(h_ap, e[:], -1.0, r[:],
                                               ADD, MIN)

            hrT = pp.tile([128, JC], BF16, name="hrT")
            celu(ps_hr[:], br_ap, hrT[:], JC, "hr")
            hlT = pp.tile([128, N], BF16, name="hlT")
            celu(ps_hl[:, 0:256], bl_ap, hlT[:, 0:256], 256, "hl0")

            # ---- v2[h, (j,o)] og-major ----
            v2f = [pp.tile([128, 512], BF16, name=f"v2f{og}")
                   for og in range(2)]
            ps_vs = []
            for og in range(2):
                ps_vo = ps_v.tile([128, 512], F32, name="ps_vo")
                ps_vs.append(ps_vo)
                for ol in range(4):
                    o = og * 4 + ol
                    nc.tensor.matmul(
                        ps_vo[:, ol * JC:(ol + 1) * JC],
                        W_sb[:, o * H:(o + 1) * H], hrT[:],
                        start=True, stop=True,
                    )

            def cast_og(og):
                src = ps_vs[og][:].rearrange("p (o j) -> p j o", o=4)
                dst = v2f[og][:].rearrange("p (j o) -> p j o", o=4)
                nc.vector.tensor_copy(dst[:, 0:64, :], src[:, 0:64, :])
                nc.scalar.copy(dst[:, 64:128, :], src[:, 64:128, :])

            cast_og(0)
            celu(ps_hl[:, 256:512], bl_ap, hlT[:, 256:512], 256, "hl1")
            cast_og(1)

            # ---- main: out[i, og*512 + j*4 + ol] ----
            evict = [nc.vector.tensor_copy, nc.scalar.copy]
            out_sbs = {}
            for k in range(8):
                ic, og = k // 2, k % 2
                if og == 0:
                    out_sbs[ic] = op.tile([128, 1024], BF16, name="out_sb")
                out_sb = out_sbs[ic]
                ps_m = ps_main.tile([128, 512], F32, name="ps_m")
                nc.tensor.matmul(
                    ps_m[:], hlT[:, ic * 128:(ic + 1) * 128], v2f[og][:],
                    start=True, stop=True,
                )
                evict[k % 2](out_sb[:, og * 512:(og + 1) * 512], ps_m[:])
                if og == 1:
                    # alternate rings: per-ring descgen serializes ~840ns/DMA
                    ring = nc.sync if ic % 2 == 0 else nc.scalar
                    ring.dma_start(out_d[ic * 128:(ic + 1) * 128, :],
                                   out_sb[:])

    _legalize_waits(nc)
    return nc


def _legalize_waits(nc):
    """walrus's per-instruction HW structs carry at most ONE sync wait.
    Split any instruction with >1 on_wait into same-engine single-wait
    EventSemaphore predecessors (engine executes them in program order)."""
    n = 0
    for bb in nc.main_func.blocks:
        insts = list(bb.instructions)
        out = []
        for ins in insts:
            si = ins.sync_info
            waits = list(si.on_wait) if si and si.on_wait else []
            if len(waits) > 1:
                for w in waits[:-1]:
                    n += 1
                    out.append(mybir.InstEventSemaphore(
                        name=f"wait-split-{n}",
                        opcode="EventSemaphore",
                        engine=ins.engine,
                        ins=[], outs=[],
                        sync_info=mybir.SyncInfo(on_wait=[w], on_update=[]),
                    ))
                si.on_wait = [waits[-1]]
            out.append(ins)
        if n:
            bb.instructions = out
    return nc


_NC_CACHE = None


def _get_nc():
    global _NC_CACHE
    if _NC_CACHE is None:
        _NC_CACHE = build_nc()
    return _NC_CACHE


def _prep_core_inputs(x_l, x_r, fc_l_W, fc_l_b, fc_r_W, fc_r_b,
                      bilinear_W, bilinear_b):
    """Host-side sharding: build the 8 per-core input dicts."""
    import ml_dtypes

    f32 = np.float32
    bf16 = ml_dtypes.bfloat16
    x_l = np.ascontiguousarray(x_l, f32)
    x_r = np.ascontiguousarray(x_r, f32)
    flW = np.asarray(fc_l_W, f32)
    frW = np.asarray(fc_r_W, f32)

    # WT[g, o*H + h] = W[o, h, g]
    WT = np.ascontiguousarray(
        np.asarray(bilinear_W, f32).transpose(2, 0, 1).reshape(128, O * H)
    ).astype(bf16)

    # Da: stationaries + hr moving + biases (identical weights per core)
    Da0 = np.zeros((128, _DA_W), bf16)

    def stat(w, c_hi, c_lo):
        """[w_{c_hi}.T ; w_{c_lo}.T] as [128,128] bf16 (None -> zeros)."""
        s = np.zeros((128, 128), f32)
        if c_hi is not None:
            s[:NIN] = w[:, c_hi * NIN:(c_hi + 1) * NIN].T
        if c_lo is not None:
            s[NIN:] = w[:, c_lo * NIN:(c_lo + 1) * NIN].T
        return s.astype(bf16)

    Da0[:, _S1_HR:_S1_HR + 128] = stat(frW, 1, 0)
    Da0[:, _S2_HR:_S2_HR + 128] = stat(frW, 2, None)
    Da0[:, _S1_HL:_S1_HL + 128] = stat(flW, 1, 0)
    Da0[:, _S2_HL:_S2_HL + 128] = stat(flW, 2, None)
    u2 = Da0.view(np.uint16)
    u2[:, _BR:_BR + 2] = np.asarray(fc_r_b, f32).reshape(-1, 1).view('<u2')
    u2[:, _BL:_BL + 2] = np.asarray(fc_l_b, f32).reshape(-1, 1).view('<u2')

    # Db per batch: rows 0:64 x_l at col offset 2, rows 64:128 at offset 1
    Dbs = []
    for b in range(B):
        Db = np.zeros((128, _DB_W), bf16)
        Db[:NIN, 2:2 + N] = x_l[b].T.astype(bf16)
        Db[NIN:, 1:1 + N] = x_l[b].T.astype(bf16)
        Dbs.append(Db)

    in_maps = []
    for core in range(N_CORES):
        b, jg = core // 4, core % 4
        j0 = jg * JC
        Da = Da0.copy()
        # rows 0:64: col _RJ+k = x_r[b, j0-1+k], k=0..129, zero-clipped
        lo = max(j0 - 1, 0)
        hi = min(j0 + JC + 1, N)
        Da[:NIN, _RJ + lo - (j0 - 1):_RJ + hi - (j0 - 1)] = \
            x_r[b, lo:hi].T.astype(bf16)
        # rows 64:128: col _RJ+2+j = x_l[b, j0+j]
        Da[NIN:, _RJ + 2:_RJ + 2 + JC] = \
            x_l[b, j0:j0 + JC].T.astype(bf16)
        in_maps.append({"Da": Da, "Db": Dbs[b], "Wt": WT})
    return in_maps


def _run(inputs, trace=False, **kw):
    nc = _get_nc()
    in_maps = _prep_core_inputs(**inputs)
    res = run_bass_kernel_spmd(
        nc, in_maps, core_ids=list(range(N_CORES)), trace=trace, **kw)
    bb = np.asarray(inputs["bilinear_b"], np.float32)
    out = np.empty((B, N, N, O), np.float32)
    for core in range(N_CORES):
        b, jg = core // 4, core % 4
        j0 = jg * JC
        # device col = og*512 + j*4 + ol  ->  [i, j, og*4+ol]
        arr = np.asarray(res.results[core]["out"]).astype(np.float32)
        arr = arr.reshape(N, 2, JC, 4).transpose(0, 2, 1, 3).reshape(N, JC, O)
        out[b, :, j0:j0 + JC, :] = arr + bb
    return out, res


def kernel(**inputs):
    out, _ = _run(inputs, trace=False)
    return out


# revision 17
# speedup vs baseline: 1.0543x; 1.0180x over previous
"""Trainium2 Bass kernel for nn_BilinearPairedLayer (v2).

out[b,i,j,o] = hl[b,i] @ W[o] @ hr[b,j] + bb[o]
  hl = celu(zl @ fc_l_W^T + fc_l_b), hr = celu(zr @ fc_r_W^T + fc_r_b)
  zl = [x_l, x_l[i-1], x_l[i+1]], zr = [x_l, x_r[j+1], x_r[j-1]]  (192 feats;
  zr[0]=x_l reproduces the torch-source init bug).

Shapes: B=2, N=512, n_in=64, H=128, n_out=8 -> out [2,512,512,8] f32.
Sharding: 8 cores = (b in {0,1}) x (j-chunk in {0..3} of 128 columns).

v2 design notes (vs the 28.9us v1):
- 64-deep contraction packing: each layer-1 shift contracts only 64
  features, so two shifts share one 128-partition matmul (stationary
  [w_c1; w_c0], moving [x_c1; x_c0]); the third shift rides a stationary
  with a zeroed lower half that kills the garbage moving rows.  6 matmuls
  -> 4, and 1920 -> 1280 PE columns.
- celu(z) = min(exp(z)-1, relu(z)) == max(z,0)+min(exp(z)-1,0): one DVE
  tensor_scalar (relu+bias), one ACT exp (bias fused), one DVE
  scalar_tensor_tensor; ~3 ops instead of 4 with a shorter chain.
- PE p-state ramp: the tensor engine runs 0.65/1.2/2.4 GHz (cold/warm/
  >=3us continuously busy).  Dummy matmuls on zeroed scratch fill the
  input-DMA wait so real matmuls inherit a hot clock.
- bf16 output + host-side bias add: halves the output DMA bytes and turns
  evictions into pure psum->bf16 casts that spread over DVE/ACT/GpSimd.
- og-major output layout (col = og*512 + j*4 + ol): a main matmul only
  waits on its own o-group's v2 cast; host transposes back.
- input split across both HWDGE rings: sync ring Da (weights+hr moving)
  then Wt; ACT ring Db (hl moving); output per-ic [128,1024]bf16 tiles
  alternate rings.

walrus's per-instruction HW structs carry at most ONE sync wait; a
post-pass splits multi-wait instructions into single-wait EventSemaphore
predecessors.
"""

import os

import numpy as np

import concourse.bass as bass
import concourse.mybir as mybir
import concourse.tile as tile
from concourse.bass_utils import run_bass_kernel_spmd

F32 = mybir.dt.float32
BF16 = mybir.dt.bfloat16
ADD = mybir.AluOpType.add
MAX = mybir.AluOpType.max
MIN = mybir.AluOpType.min

B = 2
N = 512
NIN = 64
H = 128
O = 8
JC = 128  # j-chunk per core
N_CORES = 8

N_WARM = int(os.environ.get("BK_WARM", "10"))

# Da column offsets (bf16).  Bias cols hold raw f32 bytes (bitcast back).
_S1_HR = 0      # [frT_1 ; frT_0]
_S2_HR = 128    # [frT_2 ; 0]
_S1_HL = 256    # [flT_1 ; flT_0]
_S2_HL = 384    # [flT_2 ; 0]
_RJ = 512       # rows 0:64 xr window, rows 64:128 xlj (132 cols)
_BR = 644       # fc_r_b (2 cols raw f32)
_BL = 646       # fc_l_b
_DA_W = 648
_DB_W = 516     # rows 0:64 x_l at cols 2:514, rows 64:128 x_l at cols 1:513


def build_nc():
    nc = bass.Bass("TRN2")

    Da_d = nc.dram_tensor("Da", [128, _DA_W], BF16, kind="ExternalInput")
    Db_d = nc.dram_tensor("Db", [128, _DB_W], BF16, kind="ExternalInput")
    Wt_d = nc.dram_tensor("Wt", [128, O * H], BF16, kind="ExternalInput")
    out_d = nc.dram_tensor("out", [N, JC * O], BF16, kind="ExternalOutput")

    with tile.TileContext(nc) as tc:
        with (
            tc.tile_pool(name="persist", bufs=1) as pp,
            tc.tile_pool(name="scratch", bufs=2) as sp,
            tc.tile_pool(name="outbuf", bufs=4) as op,
            tc.tile_pool(name="ps_l1", bufs=1, space="PSUM") as ps_l1,
            tc.tile_pool(name="ps_v", bufs=2, space="PSUM") as ps_v,
            tc.tile_pool(name="ps_main", bufs=4, space="PSUM") as ps_main,
        ):
            # zeroed scratch: PE pre-ramp matmul fodder + ACT warm src
            sc = pp.tile([128, 256], BF16, name="sc")
            nc.vector.memset(sc[:], 0.0)
            td = pp.tile([1, 2], F32, name="td")
            nc.vector.memset(td[0:1, 0:1], 0.0)

            Da_sb = pp.tile([128, _DA_W], BF16, name="Da_sb")
            Db_sb = pp.tile([128, _DB_W], BF16, name="Db_sb")
            W_sb = pp.tile([128, O * H], BF16, name="W_sb")
            # one DMA per ring first (the 2nd DMA on a ring pays ~1us of
            # serialized descriptor-generation latency): Da=sync, Db=ACT
            nc.sync.dma_start(Da_sb[:], Da_d[:])
            nc.scalar.dma_start(Db_sb[:], Db_d[:])
            nc.sync.dma_start(W_sb[:], Wt_d[:])
            # lazy PWP activation-table load (~1.3us) after the ACT-ring
            # DMA issue, during the input-DMA wait.
            nc.scalar.activation(td[0:1, 1:2], td[0:1, 0:1],
                                 mybir.ActivationFunctionType.Exp)

            br_ap = Da_sb[:, _BR:_BR + 2].bitcast(F32)
            bl_ap = Da_sb[:, _BL:_BL + 2].bitcast(F32)

            # ---- PE pre-ramp: dummy matmuls on zeros ----
            ps_w = ps_main.tile([128, 512], F32, name="ps_m")
            for _ in range(N_WARM):
                nc.tensor.matmul(ps_w[:, 0:256], sc[:, 0:128], sc[:],
                                 start=True, stop=True)

            # ---- layer 1 matmuls (64-deep packed) ----
            # hr[g,j]: mm1 = frT_1.T @ xr[j+1] (+) frT_0.T @ xlj;
            #          mm2 = frT_2.T @ xr[j-1] (lower stationary half = 0)
            ps_hr = ps_l1.tile([128, JC], F32, name="ps_hr")
            nc.tensor.matmul(ps_hr[:], Da_sb[:, _S1_HR:_S1_HR + 128],
                             Da_sb[:, _RJ + 2:_RJ + 2 + JC],
                             start=True, stop=False)
            nc.tensor.matmul(ps_hr[:], Da_sb[:, _S2_HR:_S2_HR + 128],
                             Da_sb[:, _RJ:_RJ + JC],
                             start=False, stop=True)
            # hl[h,i]: mm1 = flT_1.T @ x[i-1] (+) flT_0.T @ x[i];
            #          mm2 = flT_2.T @ x[i+1]
            ps_hl = ps_l1.tile([128, N], F32, name="ps_hl")
            nc.tensor.matmul(ps_hl[:], Da_sb[:, _S1_HL:_S1_HL + 128],
                             Db_sb[:, 1:1 + N], start=True, stop=False)
            nc.tensor.matmul(ps_hl[:], Da_sb[:, _S2_HL:_S2_HL + 128],
                             Db_sb[:, 3:3 + N], start=False, stop=True)

            # ---- celu: h = max(z, min(exp(z)-1, 0)), z = psum + bias ----
            # z' is the SOLE psum reader (cross-engine readers of one psum
            # serialize); s runs at DVE 4x (all-SBUF bf16), h at 2x.
            def celu(ps_ap, bias_ap, h_ap, w, tag):
                z = pp.tile([128, w], BF16, name=f"z_{tag}")
                e = pp.tile([128, w], BF16, name=f"e_{tag}")
                s = pp.tile([128, w], BF16, name=f"s_{tag}")
                nc.vector.tensor_scalar(z[:], ps_ap, bias_ap, 0.0,
                                        ADD, mybir.AluOpType.bypass)
                nc.scalar.activation(e[:], z[:],
                                     mybir.ActivationFunctionType.Exp)
                nc.vector.tensor_scalar(s[:], e[:], -1.0, 0.0, ADD, MIN)
                nc.vector.tensor_tensor(h_ap, z[:], s[:], MAX)

            hrT = pp.tile([128, JC], BF16, name="hrT")
            celu(ps_hr[:], br_ap, hrT[:], JC, "hr")
            hlT = pp.tile([128, N], BF16, name="hlT")
            celu(ps_hl[:, 0:256], bl_ap, hlT[:, 0:256], 256, "hl0")

            # ---- v2[h, (j,o)] og-major ----
            v2f = [pp.tile([128, 512], BF16, name=f"v2f{og}")
                   for og in range(2)]
            ps_vs = []
            for og in range(2):
                ps_vo = ps_v.tile([128, 512], F32, name="ps_vo")
                ps_vs.append(ps_vo)
                for ol in range(4):
                    o = og * 4 + ol
                    nc.tensor.matmul(
                        ps_vo[:, ol * JC:(ol + 1) * JC],
                        W_sb[:, o * H:(o + 1) * H], hrT[:],
                        start=True, stop=True,
                    )

            def cast_og(og):
                # one op per og (two engines on one psum serialize anyway)
                src = ps_vs[og][:].rearrange("p (o j) -> p j o", o=4)
                dst = v2f[og][:].rearrange("p (j o) -> p j o", o=4)
                if og == 0:
                    nc.vector.tensor_copy(dst, src)
                else:
                    nc.scalar.copy(dst, src)

            cast_og(0)
            celu(ps_hl[:, 256:512], bl_ap, hlT[:, 256:512], 256, "hl1")
            cast_og(1)

            # ---- main: out[i, og*512 + j*4 + ol] ----
            evict = [nc.vector.tensor_copy, nc.scalar.copy]
            out_sbs = {}
            for k in range(8):
                ic, og = k // 2, k % 2
                if og == 0:
                    out_sbs[ic] = op.tile([128, 1024], BF16, name="out_sb")
                out_sb = out_sbs[ic]
                ps_m = ps_main.tile([128, 512], F32, name="ps_m")
                nc.tensor.matmul(
                    ps_m[:], hlT[:, ic * 128:(ic + 1) * 128], v2f[og][:],
                    start=True, stop=True,
                )
                evict[k % 2](out_sb[:, og * 512:(og + 1) * 512], ps_m[:])
                if og == 1:
                    # alternate rings: per-ring descgen serializes ~840ns/DMA
                    ring = nc.sync if ic % 2 == 0 else nc.scalar
                    ring.dma_start(out_d[ic * 128:(ic + 1) * 128, :],
                                   out_sb[:])

    _legalize_waits(nc)
    return nc


def _legalize_waits(nc):
    """walrus's per-instruction HW structs carry at most ONE sync wait.
    Split any instruction with >1 on_wait into same-engine single-wait
    EventSemaphore predecessors (engine executes them in program order)."""
    n = 0
    for bb in nc.main_func.blocks:
        insts = list(bb.instructions)
        out = []
        for ins in insts:
            si = ins.sync_info
            waits = list(si.on_wait) if si and si.on_wait else []
            if len(waits) > 1:
                for w in waits[:-1]:
                    n += 1
                    out.append(mybir.InstEventSemaphore(
                        name=f"wait-split-{n}",
                        opcode="EventSemaphore",
                        engine=ins.engine,
                        ins=[], outs=[],
                        sync_info=mybir.SyncInfo(on_wait=[w], on_update=[]),
                    ))
                si.on_wait = [waits[-1]]
            out.append(ins)
        if n:
            bb.instructions = out
    return nc


_NC_CACHE = None


def _get_nc():
    global _NC_CACHE
    if _NC_CACHE is None:
        _NC_CACHE = build_nc()
    return _NC_CACHE


def _prep_core_inputs(x_l, x_r, fc_l_W, fc_l_b, fc_r_W, fc_r_b,
                      bilinear_W, bilinear_b):
    """Host-side sharding: build the 8 per-core input dicts."""
    import ml_dtypes

    f32 = np.float32
    bf16 = ml_dtypes.bfloat16
    x_l = np.ascontiguousarray(x_l, f32)
    x_r = np.ascontiguousarray(x_r, f32)
    flW = np.asarray(fc_l_W, f32)
    frW = np.asarray(fc_r_W, f32)

    # WT[g, o*H + h] = W[o, h, g]
    WT = np.ascontiguousarray(
        np.asarray(bilinear_W, f32).transpose(2, 0, 1).reshape(128, O * H)
    ).astype(bf16)

    # Da: stationaries + hr moving + biases (identical weights per core)
    Da0 = np.zeros((128, _DA_W), bf16)

    def stat(w, c_hi, c_lo):
        """[w_{c_hi}.T ; w_{c_lo}.T] as [128,128] bf16 (None -> zeros)."""
        s = np.zeros((128, 128), f32)
        if c_hi is not None:
            s[:NIN] = w[:, c_hi * NIN:(c_hi + 1) * NIN].T
        if c_lo is not None:
            s[NIN:] = w[:, c_lo * NIN:(c_lo + 1) * NIN].T
        return s.astype(bf16)

    Da0[:, _S1_HR:_S1_HR + 128] = stat(frW, 1, 0)
    Da0[:, _S2_HR:_S2_HR + 128] = stat(frW, 2, None)
    Da0[:, _S1_HL:_S1_HL + 128] = stat(flW, 1, 0)
    Da0[:, _S2_HL:_S2_HL + 128] = stat(flW, 2, None)
    u2 = Da0.view(np.uint16)
    u2[:, _BR:_BR + 2] = np.asarray(fc_r_b, f32).reshape(-1, 1).view('<u2')
    u2[:, _BL:_BL + 2] = np.asarray(fc_l_b, f32).reshape(-1, 1).view('<u2')

    # Db per batch: rows 0:64 x_l at col offset 2, rows 64:128 at offset 1
    Dbs = []
    for b in range(B):
        Db = np.zeros((128, _DB_W), bf16)
        Db[:NIN, 2:2 + N] = x_l[b].T.astype(bf16)
        Db[NIN:, 1:1 + N] = x_l[b].T.astype(bf16)
        Dbs.append(Db)

    in_maps = []
    for core in range(N_CORES):
        b, jg = core // 4, core % 4
        j0 = jg * JC
        Da = Da0.copy()
        # rows 0:64: col _RJ+k = x_r[b, j0-1+k], k=0..129, zero-clipped
        lo = max(j0 - 1, 0)
        hi = min(j0 + JC + 1, N)
        Da[:NIN, _RJ + lo - (j0 - 1):_RJ + hi - (j0 - 1)] = \
            x_r[b, lo:hi].T.astype(bf16)
        # rows 64:128: col _RJ+2+j = x_l[b, j0+j]
        Da[NIN:, _RJ + 2:_RJ + 2 + JC] = \
            x_l[b, j0:j0 + JC].T.astype(bf16)
        in_maps.append({"Da": Da, "Db": Dbs[b], "Wt": WT})
    return in_maps


def _run(inputs, trace=False, **kw):
    nc = _get_nc()
    in_maps = _prep_core_inputs(**inputs)
    res = run_bass_kernel_spmd(
        nc, in_maps, core_ids=list(range(N_CORES)), trace=trace, **kw)
    bb = np.asarray(inputs["bilinear_b"], np.float32)
    out = np.empty((B, N, N, O), np.float32)
    for core in range(N_CORES):
        b, jg = core // 4, core % 4
        j0 = jg * JC
        # device col = og*512 + j*4 + ol  ->  [i, j, og*4+ol]
        arr = np.asarray(res.results[core]["out"]).astype(np.float32)
        arr = arr.reshape(N, 2, JC, 4).transpose(0, 2, 1, 3).reshape(N, JC, O)
        out[b, :, j0:j0 + JC, :] = arr + bb
    return out, res


def kernel(**inputs):
    out, _ = _run(inputs, trace=False)
    return out


# revision 25
# speedup vs baseline: 1.0579x; 1.0034x over previous
"""Trainium2 Bass kernel for nn_BilinearPairedLayer (v2).

out[b,i,j,o] = hl[b,i] @ W[o] @ hr[b,j] + bb[o]
  hl = celu(zl @ fc_l_W^T + fc_l_b), hr = celu(zr @ fc_r_W^T + fc_r_b)
  zl = [x_l, x_l[i-1], x_l[i+1]], zr = [x_l, x_r[j+1], x_r[j-1]]  (192 feats;
  zr[0]=x_l reproduces the torch-source init bug).

Shapes: B=2, N=512, n_in=64, H=128, n_out=8 -> out [2,512,512,8] f32.
Sharding: 8 cores = (b in {0,1}) x (j-chunk in {0..3} of 128 columns).

v2 design notes (vs the 28.9us v1):
- 64-deep contraction packing: each layer-1 shift contracts only 64
  features, so two shifts share one 128-partition matmul (stationary
  [w_c1; w_c0], moving [x_c1; x_c0]); the third shift rides a stationary
  with a zeroed lower half that kills the garbage moving rows.  6 matmuls
  -> 4, and 1920 -> 1280 PE columns.
- celu(z) = min(exp(z)-1, relu(z)) == max(z,0)+min(exp(z)-1,0): one DVE
  tensor_scalar (relu+bias), one ACT exp (bias fused), one DVE
  scalar_tensor_tensor; ~3 ops instead of 4 with a shorter chain.
- PE p-state ramp: the tensor engine runs 0.65/1.2/2.4 GHz (cold/warm/
  >=3us continuously busy).  Dummy matmuls on zeroed scratch fill the
  input-DMA wait so real matmuls inherit a hot clock.
- bf16 output + host-side bias add: halves the output DMA bytes and turns
  evictions into pure psum->bf16 casts that spread over DVE/ACT/GpSimd.
- og-major output layout (col = og*512 + j*4 + ol): a main matmul only
  waits on its own o-group's v2 cast; host transposes back.
- input split across both HWDGE rings: sync ring Da (weights+hr moving)
  then Wt; ACT ring Db (hl moving); output per-ic [128,1024]bf16 tiles
  alternate rings.

walrus's per-instruction HW structs carry at most ONE sync wait; a
post-pass splits multi-wait instructions into single-wait EventSemaphore
predecessors.
"""

import os

import numpy as np

import concourse.bass as bass
import concourse.mybir as mybir
import concourse.tile as tile
from concourse.bass_utils import run_bass_kernel_spmd

F32 = mybir.dt.float32
BF16 = mybir.dt.bfloat16
ADD = mybir.AluOpType.add
MAX = mybir.AluOpType.max
MIN = mybir.AluOpType.min

B = 2
N = 512
NIN = 64
H = 128
O = 8
JC = 128  # j-chunk per core
N_CORES = 8

N_WARM = int(os.environ.get("BK_WARM", "10"))
N_WARM2 = int(os.environ.get("BK_WARM2", "3"))

# Da1 (hr inputs) column offsets (bf16).  Bias cols hold raw f32 bytes.
_S1_HR = 0      # [frT_1 ; frT_0]
_S2_HR = 128    # [frT_2 ; 0]
_RJ = 256       # rows 0:64 xr window, rows 64:128 xlj (132 cols)
_BR = 388       # fc_r_b (2 cols raw f32)
_DA1_W = 390
# Da2 (hl weights)
_S1_HL = 0      # [flT_1 ; flT_0]
_S2_HL = 128    # [flT_2 ; 0]
_BL = 256       # fc_l_b
_DA2_W = 258
_DB_W = 516     # rows 0:64 x_l at cols 2:514, rows 64:128 x_l at cols 1:513


def build_nc():
    nc = bass.Bass("TRN2")

    Da1_d = nc.dram_tensor("Da1", [128, _DA1_W], BF16, kind="ExternalInput")
    Da2_d = nc.dram_tensor("Da2", [128, _DA2_W], BF16, kind="ExternalInput")
    Db_d = nc.dram_tensor("Db", [128, _DB_W], BF16, kind="ExternalInput")
    Wt_d = nc.dram_tensor("Wt", [128, O * H], BF16, kind="ExternalInput")
    out_d = nc.dram_tensor("out", [N, JC * O], BF16, kind="ExternalOutput")

    with tile.TileContext(nc) as tc:
        with (
            tc.tile_pool(name="persist", bufs=1) as pp,
            tc.tile_pool(name="scratch", bufs=2) as sp,
            tc.tile_pool(name="outbuf", bufs=4) as op,
            tc.tile_pool(name="ps_l1", bufs=1, space="PSUM") as ps_l1,
            tc.tile_pool(name="ps_v", bufs=2, space="PSUM") as ps_v,
            tc.tile_pool(name="ps_main", bufs=4, space="PSUM") as ps_main,
        ):
            # zeroed scratch: PE pre-ramp matmul fodder + ACT warm src
            sc = pp.tile([128, 256], BF16, name="sc")
            nc.vector.memset(sc[:], 0.0)
            td = pp.tile([1, 2], F32, name="td")
            nc.vector.memset(td[0:1, 0:1], 0.0)

            Da1_sb = pp.tile([128, _DA1_W], BF16, name="Da1_sb")
            Da2_sb = pp.tile([128, _DA2_W], BF16, name="Da2_sb")
            Db_sb = pp.tile([128, _DB_W], BF16, name="Db_sb")
            W_sb = pp.tile([128, O * H], BF16, name="W_sb")
            # per-ring order (2nd+ DMA on a ring pays serialized descgen):
            # sync: Da1 (hr, small, first) -> Da2 (hl stats) -> Wt
            # ACT:  Db (hl moving)
            nc.sync.dma_start(Da1_sb[:], Da1_d[:])
            nc.scalar.dma_start(Db_sb[:], Db_d[:])
            nc.sync.dma_start(Da2_sb[:], Da2_d[:])
            nc.sync.dma_start(W_sb[:], Wt_d[:])
            # lazy PWP activation-table load (~1.3us) after the ACT-ring
            # DMA issue, during the input-DMA wait.
            nc.scalar.activation(td[0:1, 1:2], td[0:1, 0:1],
                                 mybir.ActivationFunctionType.Exp)

            br_ap = Da1_sb[:, _BR:_BR + 2].bitcast(F32)
            bl_ap = Da2_sb[:, _BL:_BL + 2].bitcast(F32)

            # ---- PE pre-ramp: dummy matmuls on zeros ----
            ps_w = ps_main.tile([128, 512], F32, name="ps_m")
            for _ in range(N_WARM):
                nc.tensor.matmul(ps_w[:, 0:256], sc[:, 0:128], sc[:],
                                 start=True, stop=True)

            # ---- layer 1 matmuls (64-deep packed) ----
            # hr[g,j]: mm1 = frT_1.T @ xr[j+1] (+) frT_0.T @ xlj;
            #          mm2 = frT_2.T @ xr[j-1] (lower stationary half = 0)
            ps_hr = ps_l1.tile([128, JC], F32, name="ps_hr")
            nc.tensor.matmul(ps_hr[:], Da1_sb[:, _S1_HR:_S1_HR + 128],
                             Da1_sb[:, _RJ + 2:_RJ + 2 + JC],
                             start=True, stop=False)
            nc.tensor.matmul(ps_hr[:], Da1_sb[:, _S2_HR:_S2_HR + 128],
                             Da1_sb[:, _RJ:_RJ + JC],
                             start=False, stop=True)
            # gap fillers: keep the PE p-state streak alive while Da2/Db land
            for _ in range(N_WARM2):
                nc.tensor.matmul(ps_w[:, 0:256], sc[:, 0:128], sc[:],
                                 start=True, stop=True)
            # hl[h,i]: mm1 = flT_1.T @ x[i-1] (+) flT_0.T @ x[i];
            #          mm2 = flT_2.T @ x[i+1]
            ps_hl = ps_l1.tile([128, N], F32, name="ps_hl")
            nc.tensor.matmul(ps_hl[:], Da2_sb[:, _S1_HL:_S1_HL + 128],
                             Db_sb[:, 1:1 + N], start=True, stop=False)
            nc.tensor.matmul(ps_hl[:], Da2_sb[:, _S2_HL:_S2_HL + 128],
                             Db_sb[:, 3:3 + N], start=False, stop=True)

            # ---- celu: h = max(z, min(exp(z)-1, 0)), z = psum + bias ----
            # z' is the SOLE psum reader (cross-engine readers of one psum
            # serialize); s runs at DVE 4x (all-SBUF bf16), h at 2x.
            def celu(ps_ap, bias_ap, h_ap, w, tag):
                z = pp.tile([128, w], BF16, name=f"z_{tag}")
                e = pp.tile([128, w], BF16, name=f"e_{tag}")
                s = pp.tile([128, w], BF16, name=f"s_{tag}")
                nc.vector.tensor_scalar(z[:], ps_ap, bias_ap, 0.0,
                                        ADD, mybir.AluOpType.bypass)
                nc.scalar.activation(e[:], z[:],
                                     mybir.ActivationFunctionType.Exp)
                nc.vector.tensor_scalar(s[:], e[:], -1.0, 0.0, ADD, MIN)
                nc.vector.tensor_tensor(h_ap, z[:], s[:], MAX)

            hrT = pp.tile([128, JC], BF16, name="hrT")
            celu(ps_hr[:], br_ap, hrT[:], JC, "hr")
            hlT = pp.tile([128, N], BF16, name="hlT")
            celu(ps_hl[:, 0:256], bl_ap, hlT[:, 0:256], 256, "hl0")

            # ---- v2[h, (j,o)] og-major ----
            v2f = [pp.tile([128, 512], BF16, name=f"v2f{og}")
                   for og in range(2)]
            ps_vs = []
            for og in range(2):
                ps_vo = ps_v.tile([128, 512], F32, name="ps_vo")
                ps_vs.append(ps_vo)
                for ol in range(4):
                    o = og * 4 + ol
                    nc.tensor.matmul(
                        ps_vo[:, ol * JC:(ol + 1) * JC],
                        W_sb[:, o * H:(o + 1) * H], hrT[:],
                        start=True, stop=True,
                    )

            def cast_og(og):
                # one op per og (two engines on one psum serialize anyway)
                src = ps_vs[og][:].rearrange("p (o j) -> p j o", o=4)
                dst = v2f[og][:].rearrange("p (j o) -> p j o", o=4)
                if og == 0:
                    nc.vector.tensor_copy(dst, src)
                else:
                    nc.scalar.copy(dst, src)

            cast_og(0)
            celu(ps_hl[:, 256:512], bl_ap, hlT[:, 256:512], 256, "hl1")
            cast_og(1)

            # ---- main: out[i, og*512 + j*4 + ol] ----
            evict = [nc.vector.tensor_copy, nc.scalar.copy]
            # last chunk on DVE so ACT is free for its own tail work;
            # ic3's issue goes to the idle sync ring for the same reason.
            ev_map = [0, 1, 0, 1, 0, 1, 1, 0]
            ring_map = [nc.sync, nc.scalar, nc.scalar, nc.sync]
            out_sbs = {}
            for k in range(8):
                ic, og = k // 2, k % 2
                if og == 0:
                    out_sbs[ic] = op.tile([128, 1024], BF16, name="out_sb")
                out_sb = out_sbs[ic]
                ps_m = ps_main.tile([128, 512], F32, name="ps_m")
                nc.tensor.matmul(
                    ps_m[:], hlT[:, ic * 128:(ic + 1) * 128], v2f[og][:],
                    start=True, stop=True,
                )
                evict[ev_map[k]](out_sb[:, og * 512:(og + 1) * 512], ps_m[:])
                if og == 1:
                    ring_map[ic].dma_start(out_d[ic * 128:(ic + 1) * 128, :],
                                           out_sb[:])

    _legalize_waits(nc)
    return nc


def _legalize_waits(nc):
    """walrus's per-instruction HW structs carry at most ONE sync wait.
    Split any instruction with >1 on_wait into same-engine single-wait
    EventSemaphore predecessors (engine executes them in program order)."""
    n = 0
    for bb in nc.main_func.blocks:
        insts = list(bb.instructions)
        out = []
        for ins in insts:
            si = ins.sync_info
            waits = list(si.on_wait) if si and si.on_wait else []
            if len(waits) > 1:
                for w in waits[:-1]:
                    n += 1
                    out.append(mybir.InstEventSemaphore(
                        name=f"wait-split-{n}",
                        opcode="EventSemaphore",
                        engine=ins.engine,
                        ins=[], outs=[],
                        sync_info=mybir.SyncInfo(on_wait=[w], on_update=[]),
                    ))
                si.on_wait = [waits[-1]]
            out.append(ins)
        if n:
            bb.instructions = out
    return nc


_NC_CACHE = None


def _get_nc():
    global _NC_CACHE
    if _NC_CACHE is None:
        _NC_CACHE = build_nc()
    return _NC_CACHE


def _prep_core_inputs(x_l, x_r, fc_l_W, fc_l_b, fc_r_W, fc_r_b,
                      bilinear_W, bilinear_b):
    """Host-side sharding: build the 8 per-core input dicts."""
    import ml_dtypes

    f32 = np.float32
    bf16 = ml_dtypes.bfloat16
    x_l = np.ascontiguousarray(x_l, f32)
    x_r = np.ascontiguousarray(x_r, f32)
    flW = np.asarray(fc_l_W, f32)
    frW = np.asarray(fc_r_W, f32)

    # WT[g, o*H + h] = W[o, h, g]
    WT = np.ascontiguousarray(
        np.asarray(bilinear_W, f32).transpose(2, 0, 1).reshape(128, O * H)
    ).astype(bf16)

    def stat(w, c_hi, c_lo):
        """[w_{c_hi}.T ; w_{c_lo}.T] as [128,128] bf16 (None -> zeros)."""
        s = np.zeros((128, 128), f32)
        if c_hi is not None:
            s[:NIN] = w[:, c_hi * NIN:(c_hi + 1) * NIN].T
        if c_lo is not None:
            s[NIN:] = w[:, c_lo * NIN:(c_lo + 1) * NIN].T
        return s.astype(bf16)

    # Da1: hr stationaries + hr moving + fc_r_b (per-core Rj region)
    Da10 = np.zeros((128, _DA1_W), bf16)
    Da10[:, _S1_HR:_S1_HR + 128] = stat(frW, 1, 0)
    Da10[:, _S2_HR:_S2_HR + 128] = stat(frW, 2, None)
    Da10.view(np.uint16)[:, _BR:_BR + 2] = \
        np.asarray(fc_r_b, f32).reshape(-1, 1).view('<u2')
    # Da2: hl stationaries + fc_l_b (identical across cores)
    Da2 = np.zeros((128, _DA2_W), bf16)
    Da2[:, _S1_HL:_S1_HL + 128] = stat(flW, 1, 0)
    Da2[:, _S2_HL:_S2_HL + 128] = stat(flW, 2, None)
    Da2.view(np.uint16)[:, _BL:_BL + 2] = \
        np.asarray(fc_l_b, f32).reshape(-1, 1).view('<u2')

    # Db per batch: rows 0:64 x_l at col offset 2, rows 64:128 at offset 1
    Dbs = []
    for b in range(B):
        Db = np.zeros((128, _DB_W), bf16)
        Db[:NIN, 2:2 + N] = x_l[b].T.astype(bf16)
        Db[NIN:, 1:1 + N] = x_l[b].T.astype(bf16)
        Dbs.append(Db)

    in_maps = []
    for core in range(N_CORES):
        b, jg = core // 4, core % 4
        j0 = jg * JC
        Da1 = Da10.copy()
        # rows 0:64: col _RJ+k = x_r[b, j0-1+k], k=0..129, zero-clipped
        lo = max(j0 - 1, 0)
        hi = min(j0 + JC + 1, N)
        Da1[:NIN, _RJ + lo - (j0 - 1):_RJ + hi - (j0 - 1)] = \
            x_r[b, lo:hi].T.astype(bf16)
        # rows 64:128: col _RJ+2+j = x_l[b, j0+j]
        Da1[NIN:, _RJ + 2:_RJ + 2 + JC] = \
            x_l[b, j0:j0 + JC].T.astype(bf16)
        in_maps.append({"Da1": Da1, "Da2": Da2, "Db": Dbs[b], "Wt": WT})
    return in_maps


def _run(inputs, trace=False, **kw):
    nc = _get_nc()
    in_maps = _prep_core_inputs(**inputs)
    res = run_bass_kernel_spmd(
        nc, in_maps, core_ids=list(range(N_CORES)), trace=trace, **kw)
    bb = np.asarray(inputs["bilinear_b"], np.float32)
    out = np.empty((B, N, N, O), np.float32)
    for core in range(N_CORES):
        b, jg = core // 4, core % 4
        j0 = jg * JC
        # device col = og*512 + j*4 + ol  ->  [i, j, og*4+ol]
        arr = np.asarray(res.results[core]["out"]).astype(np.float32)
        arr = arr.reshape(N, 2, JC, 4).transpose(0, 2, 1, 3).reshape(N, JC, O)
        out[b, :, j0:j0 + JC, :] = arr + bb
    return out, res


def kernel(**inputs):
    out, _ = _run(inputs, trace=False)
    return out
